# revision 22
# baseline (speedup 1.0000x reference)
"""Trainium2 Bass kernel: per-pixel channel shuffle + 3x3 conv (stride 1, pad 1).

Problem: x [32,256,56,56] f32, w [256,256,3,3] f32 (OIHW), perm [3136,256] i32;
out[b,:,h,w] = conv3x3(xs)[b,:,h,w] where xs[b,:,l] = x[b, perm[l,:], l].

Strategy (8 NeuronCores, data-parallel over batch, 4 batches/core):
  host: x -> pixel-major bf16 packed [B_LOC, 7, 112, 1024] (448 px per scatter
        call, 4 pixels per partition); inverse-perm int16 tables in the same
        layout; w transformed with the Winograd F(2,3) G-matrix along kw into
        a [128, 48*128] lhsT sheet (t x dh x ct x oct tiles).
  device, per batch:
    1. 7 contiguous DMAs of [112, 1024] call tiles (448 px = 8 image rows).
    2. 7 GPSIMD local_scatter calls (channels=112, num_idxs=1024) apply each
       pixel's inverse channel permutation within its partition.
    3. Per call: 8 PE transposes [112,128]->[128,112] produce [c, l] 2-row
       strips in one PSUM bank; 2 strided DVE copies place all 8 rows into a
       zero-padded 58-wide flat image xs.
    4. Winograd F(2,3) along W: DVE builds 4 transformed planes per ct
       (d0=v0-v2, d1=v1+v2, d2=v2-v1, d3=v1-v3 from stride-2 column pairs of
       xs), in 4 row-chunks chasing the scatter pipeline.
    5. GEMMs: per (oct, 14-row group) accumulate 6 matmuls (3 dh x 2 ct) into
       each of 4 M_t PSUM banks (t rotates innermost so banks alternate and
       LDWEIGHTS hides); 1.5x fewer PE cycles than direct conv.
    6. Output transform on DVE: col 2j = M0+M1+M2, col 2j+1 = M1-M2-M3,
       written bf16 into an interleaved [14,56] tile; scalar-queue DMA out;
       host upcasts to f32.
  Schedule: every batch chases its scatter calls (row-group g needs only calls
  through (7g+8+7)/8); batch b+1's scatter runs on GPSIMD while PE does batch
  b's GEMMs. Head DMAs (call-0/1 inputs split in half on sync, weights split 8
  ways on scalar) are ordered for first-scatter latency.
"""

import os
import sys
import types
import numpy as np

_STATE = {}
LAST_RESULT = None

B, C, H, W = 32, 256, 56, 56
HW = H * W          # 3136
PADW = 58
XS_LEN = 3376       # 58 rows x 58 cols + 12 slack
N_CORES = 8
B_LOC = B // N_CORES  # 4

NCALL = 7           # scatter calls per batch
PPC = 448           # pixels per call (8 image rows)
PPP = 4             # pixels per partition per call
CH = 112            # partitions (channels arg) per scatter call
NIDX = PPP * C      # 1024 idx / elems per partition per call

NT = 28             # Winograd tiles per row (56 / 2)
DPLANE = 58 * NT    # one d_t plane: 58 padded rows x 28 tiles = 1624
PL = 58 * 29        # parity plane: 58 padded rows x 29 cols (E=even, O=odd)
RG = 14             # output rows per GEMM group (n = 14*28 = 392 <= 512 PSUM)
NRG = 4
NW = RG * NT        # 392


def _install_ntff_shim():
    # antenv.axon_hooks is absent in some images; provide it so trace=True
    # (BASS_TRACE=1) can capture NTFF profiles instead of crashing.
    name = "antenv.axon_hooks"
    if name in sys.modules:
        return
    try:
        import antenv  # noqa: F401

        m = types.ModuleType(name)
        m._hook = None
        m.set_axon_ntff_profile_hook = lambda h: setattr(m, "_hook", h)
        m.get_axon_ntff_profile_hook = lambda: m._hook
        sys.modules[name] = m
        setattr(sys.modules["antenv"], "axon_hooks", m)
        from trn_agent_boot.trn_boot import _ntff_profile_via_ctypes

        hook = _ntff_profile_via_ctypes("/opt/axon/libaxon_pjrt.so")
        if hook is not None:
            m.set_axon_ntff_profile_hook(hook)
    except Exception:
        pass


def _build_kernel():
    import concourse.bass as bass
    import concourse.mybir as mybir
    from concourse import bacc, tile
    from concourse.masks import make_identity
    from contextlib import ExitStack

    F32 = mybir.dt.float32
    BF16 = mybir.dt.bfloat16
    I16 = mybir.dt.int16
    ADD = mybir.AluOpType.add
    SUB = mybir.AluOpType.subtract

    nc = bacc.Bacc("TRN2", target_bir_lowering=False, debug=False, num_devices=N_CORES)

    xb = nc.dram_tensor("xb", [B_LOC, NCALL, CH, NIDX], BF16, kind="ExternalInput")
    wt = nc.dram_tensor("wt", [128, 48 * 128], BF16, kind="ExternalInput")
    idxt = nc.dram_tensor("idxt", [CH, NCALL * NIDX], I16, kind="ExternalInput")
    out = nc.dram_tensor("out", [B_LOC, C, HW], BF16, kind="ExternalOutput")

    with tile.TileContext(nc) as tc, ExitStack() as ctx:
        const = ctx.enter_context(tc.tile_pool(name="const", bufs=1))
        xin_pool = ctx.enter_context(tc.tile_pool(name="xin", bufs=10))
        sout_pool = ctx.enter_context(tc.tile_pool(name="sout", bufs=10))

        def xin_dma(b, k, nsplit=1):
            xin = xin_pool.tile([128, NIDX], BF16, name="xin", tag="xin")
            step = NIDX // nsplit
            for c0 in range(0, NIDX, step):
                nc.sync.dma_start(
                    out=xin[0:CH, c0 : c0 + step], in_=xb[b, k, :, c0 : c0 + step]
                )
            return xin

        wsb = const.tile([128, 48 * 128], BF16)
        idxtiles = {}

        def idx_dma(k, nsplit=1):
            t = const.tile([128, NIDX], I16, name=f"idx{k}", tag=f"idx{k}")
            idxtiles[k] = t
            step = NIDX // nsplit
            for c0 in range(0, NIDX, step):
                nc.sync.dma_start(
                    out=t[0:CH, c0 : c0 + step],
                    in_=idxt[:, k * NIDX + c0 : k * NIDX + c0 + step],
                )

        # head critical path: first two calls' inputs on sync, weights on the
        # scalar queue, bulk idx tables behind.
        xin_pre = {}
        idx_dma(0, nsplit=2)
        xin_pre[(0, 0)] = xin_dma(0, 0, nsplit=2)
        idx_dma(1, nsplit=2)
        xin_pre[(0, 1)] = xin_dma(0, 1, nsplit=2)
        for q in range(8):
            nc.scalar.dma_start(
                out=wsb[:, q * 768 : (q + 1) * 768],
                in_=wt[:, q * 768 : (q + 1) * 768],
            )
        ident = const.tile([128, 128], BF16)
        make_identity(nc, ident[:, :])
        xs_pool = ctx.enter_context(tc.tile_pool(name="xs", bufs=2))
        dt_pool = ctx.enter_context(tc.tile_pool(name="dt", bufs=2))
        ost_pool = ctx.enter_context(tc.tile_pool(name="ost", bufs=4))
        tmp_pool = ctx.enter_context(tc.tile_pool(name="tmp", bufs=2))
        tps_pool = ctx.enter_context(tc.tile_pool(name="tps", bufs=2, space="PSUM"))
        mps_pool = ctx.enter_context(tc.tile_pool(name="mps", bufs=6, space="PSUM"))

        xs_tiles = {}
        dt_tiles = {}

        def shuffle_call(b, k):
            # DMA a 448-pixel tile in [partition=pixel%112, (j, c)] layout,
            # scatter channels within each partition, transpose back to [c, l]
            # and place the 2-row strips into the padded image.
            if k == 0:
                # four parity planes: (ct, par) with par 0=E (even padded
                # cols 0..56), 1=O (odd padded cols 1..57); row stride 29.
                xs = xs_pool.tile([128, 4 * PL], BF16, name="xs", tag="xs")
                xs_tiles[b] = xs
                for ct in range(2):
                    for par in range(2):
                        base = (ct * 2 + par) * PL
                        nc.vector.memset(xs[:, base : base + 29], 0.0)
                        nc.vector.memset(xs[:, base + 57 * 29 : base + PL], 0.0)
                        pcol = 0 if par == 0 else 28
                        nc.vector.memset(
                            xs[
                                :, base + 29 + pcol : base + 29 + pcol + 56 * 29
                            ].rearrange("p (r x) -> p r x", r=56)[:, :, 0:1],
                            0.0,
                        )
            xs = xs_tiles[b]

            if k not in idxtiles:
                idx_dma(k)
            xin = xin_pre.pop((b, k), None)
            if xin is None:
                xin = xin_dma(b, k)
            sout = sout_pool.tile([128, NIDX], BF16, name="sout", tag="sout")
            nc.gpsimd.local_scatter(
                out_ap=sout[0:CH, :],
                data_ap=xin[0:CH, :],
                idxs_ap=idxtiles[k][0:CH, :],
                channels=CH,
                num_elems=NIDX,
                num_idxs=NIDX,
            )
            tps = tps_pool.tile([128, 8 * CH], BF16, name="tps", tag="tps")
            # stream identity columns even-pixels-first: transpose output is
            # [c, (even 56 | odd 56)] so parity planes get contiguous copies.
            identv = ident[0:CH, 0:CH].rearrange("p (j two) -> p two j", two=2)
            for j in range(PPP):
                for ct in range(2):
                    nc.tensor.transpose(
                        tps[:, ct * 448 + j * CH : ct * 448 + (j + 1) * CH],
                        sout[0:CH, j * C + ct * 128 : j * C + ct * 128 + 128],
                        identv,
                    )
            for ct in range(2):
                src = tps[:, ct * 448 : ct * 448 + 448].rearrange(
                    "p (j h r x) -> p j h r x", j=PPP, h=2, r=2
                )
                for par in range(2):
                    # even data cols -> O plane cols 0..27; odd -> E cols 1..28
                    base = (ct * 2 + (1 - par)) * PL
                    off = base + (1 + 8 * k) * 29 + par
                    nc.vector.tensor_copy(
                        xs[:, off : off + 8 * 29].rearrange(
                            "p (j r c) -> p j r c", j=PPP, r=2
                        )[:, :, :, 0:28],
                        src[:, :, par, :, :],
                    )

        # v0=E[j], v1=O[j], v2=E[j+1], v3=O[j+1]; (par, col0) operand specs
        DSPEC = (((0, 0), (0, 1), SUB), ((1, 0), (0, 1), ADD),
                 ((0, 1), (1, 0), SUB), ((1, 0), (1, 1), SUB))

        def xs_cols(xs, ct, lo, hi, pc):
            par, j0 = pc
            base = (ct * 2 + par) * PL
            return xs[:, base + lo * 29 : base + hi * 29].rearrange(
                "p (r c) -> p r c", c=29
            )[:, :, j0 : j0 + NT]

        def dpl_chunk(b, lo, hi):
            # build d_t planes for padded rows [lo, hi)
            if lo == 0:
                dtile = dt_pool.tile([128, 8 * DPLANE], BF16, name="dt", tag="dt")
                dt_tiles[b] = dtile
            dtile = dt_tiles[b]
            xs = xs_tiles[b]
            for ct in range(2):
                for t, (a0, a1, op) in enumerate(DSPEC):
                    base = (ct * 4 + t) * DPLANE
                    dst = dtile[:, base + lo * NT : base + hi * NT].rearrange(
                        "p (r j) -> p r j", j=NT
                    )
                    nc.vector.scalar_tensor_tensor(
                        dst,
                        xs_cols(xs, ct, lo, hi, a0),
                        0.0,
                        xs_cols(xs, ct, lo, hi, a1),
                        ADD,
                        op,
                    )

        def win_chunk(b, oct, rg):
            # GEMMs for output rows [rg*14, rg*14+14), then output transform.
            dtile = dt_tiles[b]
            r0 = rg * RG
            mps = [
                mps_pool.tile([128, NW], F32, name="mp", tag="mp") for _ in range(4)
            ]
            for i6 in range(6):
                dh, ct = divmod(i6, 2)
                for t in range(4):
                    widx = ((t * 3 + dh) * 2 + ct) * 2 + oct
                    base = (ct * 4 + t) * DPLANE
                    nc.tensor.matmul(
                        mps[t][:, :],
                        lhsT=wsb[:, widx * 128 : (widx + 1) * 128],
                        rhs=dtile[:, base + (r0 + dh) * NT : base + (r0 + dh) * NT + NW],
                        start=(i6 == 0),
                        stop=(i6 == 5),
                    )
            ost = ost_pool.tile([128, RG * W], BF16, name="ost", tag="ost")
            # stage M_t to bf16 SBUF on the scalar engine so the DVE combines
            # run in fast mode (2-byte, SBUF, contiguous).
            ms = []
            for t in range(4):
                mt = tmp_pool.tile([128, NW], BF16, name=f"m{t}", tag=f"m{t}")
                nc.scalar.copy(mt[:, :], mps[t][:, :])
                ms.append(mt)
            t1 = tmp_pool.tile([128, NW], BF16, name="t1", tag="t1")
            nc.vector.scalar_tensor_tensor(t1[:, :], ms[0][:, :], 0.0, ms[1][:, :], ADD, ADD)
            nc.vector.scalar_tensor_tensor(
                ost[:, 0 : 2 * NW : 2], t1[:, :], 0.0, ms[2][:, :], ADD, ADD
            )
            t2 = tmp_pool.tile([128, NW], BF16, name="t2", tag="t2")
            nc.vector.scalar_tensor_tensor(t2[:, :], ms[1][:, :], 0.0, ms[2][:, :], ADD, SUB)
            nc.vector.scalar_tensor_tensor(
                ost[:, 1 : 2 * NW : 2], t2[:, :], 0.0, ms[3][:, :], ADD, SUB
            )
            nc.scalar.dma_start(
                out=out[b, oct * 128 : (oct + 1) * 128, r0 * W : (r0 + RG) * W],
                in_=ost[:, :],
            )

        # Chase schedule: d-plane chunk c covers padded rows; row-group rg
        # needs d rows [rg*14, rg*14+16) -> scatter calls as annotated.
        for b in range(B_LOC):
            shuffle_call(b, 0)
            shuffle_call(b, 1)
            dpl_chunk(b, 0, 16)      # needs xs rows <= 15 (calls 0-1)
            win_chunk(b, 0, 0)
            win_chunk(b, 1, 0)
            shuffle_call(b, 2)
            shuffle_call(b, 3)
            dpl_chunk(b, 16, 30)     # needs xs rows <= 29 (call 3)
            win_chunk(b, 0, 1)
            win_chunk(b, 1, 1)
            shuffle_call(b, 4)
            shuffle_call(b, 5)
            dpl_chunk(b, 30, 44)     # needs xs rows <= 43 (call 5)
            win_chunk(b, 0, 2)
            win_chunk(b, 1, 2)
            shuffle_call(b, 6)
            dpl_chunk(b, 44, 58)     # needs all rows + bottom pad
            win_chunk(b, 0, 3)
            win_chunk(b, 1, 3)

    nc.compile()
    return nc


def _host_prep(x, w, perm):
    import ml_dtypes

    # pixel-major bf16 x packed per scatter call: [B, NCALL, CH, PPP*C]
    xf = (
        x.transpose(0, 2, 3, 1)
        .reshape(B, NCALL, PPP, CH, C)
        .transpose(0, 1, 3, 2, 4)
        .reshape(B, NCALL, CH, NIDX)
        .astype(ml_dtypes.bfloat16)
    )

    # Winograd F(2,3) weight transform along kw: G = [[1,0,0],[.5,.5,.5],
    # [.5,-.5,.5],[0,0,1]]; lhsT tiles [c, o] indexed (t, dh, ct, oct).
    wf = np.asarray(w, dtype=np.float32)
    wtile = np.empty((48, 128, 128), dtype=ml_dtypes.bfloat16)
    for t in range(4):
        for dh in range(3):
            for ct in range(2):
                for oct in range(2):
                    i = ((t * 3 + dh) * 2 + ct) * 2 + oct
                    blk = wf[
                        oct * 128 : (oct + 1) * 128, ct * 128 : (ct + 1) * 128, dh, :
                    ]
                    if t == 0:
                        g = blk[:, :, 0]
                    elif t == 1:
                        g = 0.5 * (blk[:, :, 0] + blk[:, :, 1] + blk[:, :, 2])
                    elif t == 2:
                        g = 0.5 * (blk[:, :, 0] - blk[:, :, 1] + blk[:, :, 2])
                    else:
                        g = blk[:, :, 2]
                    wtile[i] = g.T.astype(ml_dtypes.bfloat16)
    wtile = np.ascontiguousarray(wtile.transpose(1, 0, 2).reshape(128, 48 * 128))

    # inverse permutation: iperm[l, c] = position of channel c in xs at pixel l
    iperm = np.empty((HW, C), dtype=np.int16)
    np.put_along_axis(
        iperm, perm.astype(np.int64), np.arange(C, dtype=np.int16)[None, :], axis=1
    )
    # scatter idx table: idxt[p, k*NIDX + j*C + c] = j*C + iperm[448k+112j+p, c]
    ip = iperm.reshape(NCALL, PPP, CH, C).transpose(2, 0, 1, 3).copy()
    ip += (np.arange(PPP, dtype=np.int16) * C)[None, None, :, None]
    idxt = np.ascontiguousarray(ip.reshape(CH, NCALL * NIDX))

    in_maps = []
    for cidx in range(N_CORES):
        in_maps.append(
            {
                "xb": np.ascontiguousarray(xf[cidx * B_LOC : (cidx + 1) * B_LOC]),
                "wt": wtile,
                "idxt": idxt,
            }
        )
    return in_maps


def kernel(x, w, perm):
    global LAST_RESULT
    _install_ntff_shim()
    from concourse.bass_utils import run_bass_kernel_spmd

    x = np.asarray(x, dtype=np.float32)
    w = np.asarray(w, dtype=np.float32)
    perm = np.asarray(perm)

    if "nc" not in _STATE:
        _STATE["nc"] = _build_kernel()
    nc = _STATE["nc"]

    in_maps = _host_prep(x, w, perm)
    res = run_bass_kernel_spmd(nc, in_maps, core_ids=list(range(N_CORES)))
    LAST_RESULT = res
    out = np.concatenate(
        [r["out"].reshape(B_LOC, C, H, W) for r in res.results], axis=0
    )
    return out.astype(np.float32)
